# revision 13
# baseline (speedup 1.0000x reference)
import sys
sys.path.insert(0, '/opt/trn_rl_repo')
import numpy as np
from contextlib import ExitStack

import concourse.bass as bass
import concourse.bacc as bacc
import concourse.tile as tile
import concourse.mybir as mybir
from concourse.bass_utils import run_bass_kernel_spmd

f32 = mybir.dt.float32
f32r = mybir.dt.float32r
AF = mybir.ActivationFunctionType
ALU = mybir.AluOpType
AX = mybir.AxisListType

B, T, C = 8, 4096, 1024
NH, HD = 64, 16
NCB = C // 128        # 8 column blocks of 128
TT = 256              # t-chunk
NCHUNK = T // TT      # 16
NTS = TT // 128       # 2 sub-tiles of 128 rows per chunk

_cache = {}


def build_nc():
    nc = bacc.Bacc("TRN2", target_bir_lowering=False, debug=False, num_devices=8)

    x_d = nc.dram_tensor("x", [T, C], f32, kind="ExternalInput")
    wq_d = nc.dram_tensor("wqT", [C, C], f32, kind="ExternalInput")
    wk_d = nc.dram_tensor("wkT", [C, C], f32, kind="ExternalInput")
    wv_d = nc.dram_tensor("wvT", [C, C], f32, kind="ExternalInput")
    wp_d = nc.dram_tensor("wpT", [C, C], f32, kind="ExternalInput")
    bqc_d = nc.dram_tensor("bq_cols", [128, NCB], f32, kind="ExternalInput")
    bk_d = nc.dram_tensor("bk_rep", [128, C], f32, kind="ExternalInput")
    bv_d = nc.dram_tensor("bv_rep", [128, C], f32, kind="ExternalInput")
    bp_d = nc.dram_tensor("bp_rep", [128, C], f32, kind="ExternalInput")
    id_d = nc.dram_tensor("ident", [128, 128], f32, kind="ExternalInput")
    e8_d = nc.dram_tensor("e8", [8, 128], f32, kind="ExternalInput")
    ones_d = nc.dram_tensor("ones1", [1, 1], f32, kind="ExternalInput")
    onescol_d = nc.dram_tensor("onescol", [128, 1], f32, kind="ExternalInput")
    m128_d = nc.dram_tensor("mask128", [128, 128], f32, kind="ExternalInput")
    e8t_d = nc.dram_tensor("e8T", [128, 8], f32, kind="ExternalInput")
    out_d = nc.dram_tensor("out", [T, C], f32, kind="ExternalOutput")

    with ExitStack() as octx:
        octx.enter_context(nc.allow_low_precision(reason="f32r mantissa rounding is intentional"))
        tc = octx.enter_context(tile.TileContext(nc))
        consts = octx.enter_context(tc.tile_pool(name="consts", bufs=1))
        dram = octx.enter_context(tc.tile_pool(name="dram", bufs=1, space="DRAM"))

        qspill = dram.tile([NCB, 128, T], f32r, tag="qspill")

        ident = consts.tile([128, 128], f32, tag="ident")
        nc.sync.dma_start(ident[:], id_d[:])
        e8 = consts.tile([8, 128], f32r, tag="e8")
        nc.sync.dma_start(e8[:], e8_d[:].bitcast(f32r))
        ones1 = consts.tile([1, 1], f32r, tag="ones1")
        nc.sync.dma_start(ones1[:], ones_d[:].bitcast(f32r))
        onesc = consts.tile([128, 1], f32r, tag="onesc")
        nc.sync.dma_start(onesc[:], onescol_d[:].bitcast(f32r))
        m128 = consts.tile([128, 128], f32, tag="m128")
        nc.sync.dma_start(m128[:], m128_d[:])
        e8t = consts.tile([128, 8], f32, tag="e8t")
        nc.sync.dma_start(e8t[:], e8t_d[:])
        bqc = consts.tile([128, NCB], f32, tag="bqc")
        nc.sync.dma_start(bqc[:], bqc_d[:])
        bkr = consts.tile([128, C], f32, tag="bkr")
        nc.sync.dma_start(bkr[:], bk_d[:])
        bvr = consts.tile([128, C], f32, tag="bvr")
        nc.sync.dma_start(bvr[:], bv_d[:])
        bpr = consts.tile([128, C], f32, tag="bpr")
        nc.sync.dma_start(bpr[:], bp_d[:])

        # block-diag context + denominator weights built between passes
        ctxd = [consts.tile([128, 128], f32r, tag=f"ctxd{g}", name=f"ctxd{g}") for g in range(NCB)]
        dg = [consts.tile([128, 8], f32r, tag=f"dg{g}", name=f"dg{g}") for g in range(NCB)]


        # ---------------- pass 1 ----------------
        with ExitStack() as p1:
            wts = p1.enter_context(tc.tile_pool(name="wts", bufs=1))
            xnp = p1.enter_context(tc.tile_pool(name="xn", bufs=2))
            xtp = p1.enter_context(tc.tile_pool(name="xt", bufs=2))
            kep = p1.enter_context(tc.tile_pool(name="ke", bufs=2))
            vp = p1.enter_context(tc.tile_pool(name="v", bufs=2))
            qp = p1.enter_context(tc.tile_pool(name="q", bufs=2))
            smallp = p1.enter_context(tc.tile_pool(name="small", bufs=2))
            ps_tq = p1.enter_context(
                tc.tile_pool(name="ps_tq", bufs=2, space="PSUM"))
            ps_kv = p1.enter_context(
                tc.tile_pool(name="ps_kv", bufs=2, space="PSUM"))
            ps_ctx = p1.enter_context(
                tc.tile_pool(name="ps_ctx", bufs=1, space="PSUM"))
            ps_cx = p1.enter_context(
                tc.tile_pool(name="ps_cx", bufs=2, space="PSUM"))
            kcum = [ps_ctx.tile([1, 512], f32, tag=f"kcum{i}", name=f"kcum{i}") for i in range(2)]
            ctxacc = [consts.tile([128, 128], f32, tag=f"cxa{g}", name=f"cxa{g}")
                      for g in range(NCB)]
            for g in range(NCB):
                nc.gpsimd.memset(ctxacc[g][:], 0.0)

            wq = [wts.tile([128, C], f32r, tag=f"wq{i}", name=f"wq{i}") for i in range(NCB)]
            wk = [wts.tile([128, C], f32r, tag=f"wk{i}", name=f"wk{i}") for i in range(NCB)]
            wv = [wts.tile([128, C], f32r, tag=f"wv{i}", name=f"wv{i}") for i in range(NCB)]
            for i in range(NCB):
                nc.sync.dma_start(wq[i][:], wq_d[i * 128:(i + 1) * 128, :].bitcast(f32r))
                nc.sync.dma_start(wk[i][:], wk_d[i * 128:(i + 1) * 128, :].bitcast(f32r))
                nc.sync.dma_start(wv[i][:], wv_d[i * 128:(i + 1) * 128, :].bitcast(f32r))

            for c in range(NCHUNK):
                t0 = c * TT
                # load + transpose x chunk
                xn = [xnp.tile([128, C], f32, tag=f"xn{ts}", name=f"xn{ts}") for ts in range(NTS)]
                for ts in range(NTS):
                    nc.sync.dma_start(
                        xn[ts][:], x_d[t0 + ts * 128: t0 + (ts + 1) * 128, :])
                xt = [xtp.tile([128, TT], f32r, tag=f"xt{i}", name=f"xt{i}") for i in range(NCB)]
                for ts in range(NTS):
                    for cb in range(NCB):
                        tp = ps_tq.tile([128, 256], f32, tag="tq")
                        nc.tensor.transpose(
                            tp[:, :128], xn[ts][:, cb * 128:(cb + 1) * 128], ident[:])
                        nc.vector.tensor_copy(
                            xt[cb][:, ts * 128:(ts + 1) * 128],
                            tp[:, :128].bitcast(f32r))

                # Q^T projection + exp, spill to DRAM
                for cb in range(NCB):
                    qps = ps_tq.tile([128, 256], f32, tag="tq")
                    for ci in range(NCB):
                        nc.tensor.matmul(
                            qps[:], wq[ci][:, cb * 128:(cb + 1) * 128], xt[ci][:],
                            start=(ci == 0), stop=(ci == NCB - 1))
                    qe = qp.tile([128, TT], f32r, tag=f"q{cb}")
                    nc.scalar.activation(
                        qe[:], qps[:], AF.Exp, bias=bqc[:, cb:cb + 1], scale=1.0)
                    nc.sync.dma_start(qspill[cb, :, t0:t0 + TT], qe[:])

                # K and V projections (natural layout), k softmax, ctx/kcum accum
                ke = [[None] * 2 for _ in range(NTS)]
                vt = [[None] * 2 for _ in range(NTS)]
                for ts in range(NTS):
                    for ch in range(2):
                        kps = ps_kv.tile([128, 512], f32, tag="kv")
                        for ci in range(NCB):
                            nc.tensor.matmul(
                                kps[:],
                                xt[ci][:, ts * 128:(ts + 1) * 128],
                                wk[ci][:, ch * 512:(ch + 1) * 512],
                                start=(ci == 0), stop=(ci == NCB - 1))
                        nc.vector.scalar_tensor_tensor(
                            kps[:], kps[:], 1.0, bkr[:, ch * 512:(ch + 1) * 512],
                            op0=ALU.mult, op1=ALU.add)
                        ket = kep.tile([128, 512], f32r, tag=f"ke{ts}{ch}")
                        nc.scalar.activation(ket[:], kps[:], AF.Exp)
                        ke3 = ket[:].rearrange("p (h d) -> p h d", d=HD)
                        ksum = smallp.tile([128, 32], f32, tag=f"ksum{ts}{ch}")
                        nc.vector.tensor_reduce(
                            ksum[:], ke3, axis=AX.X, op=ALU.add)
                        krec = smallp.tile([128, 32], f32, tag=f"krec{ts}{ch}")
                        nc.vector.reciprocal(krec[:], ksum[:])
                        nc.vector.scalar_tensor_tensor(
                            ke3, ke3, 1.0,
                            krec[:, :, None].broadcast_to([128, 32, HD]),
                            op0=ALU.mult, op1=ALU.mult)
                        ke[ts][ch] = ket

                        vps = ps_kv.tile([128, 512], f32, tag="kv")
                        for ci in range(NCB):
                            nc.tensor.matmul(
                                vps[:],
                                xt[ci][:, ts * 128:(ts + 1) * 128],
                                wv[ci][:, ch * 512:(ch + 1) * 512],
                                start=(ci == 0), stop=(ci == NCB - 1))
                        vtt = vp.tile([128, 512], f32r, tag=f"v{ts}{ch}")
                        nc.vector.scalar_tensor_tensor(
                            vtt[:], vps[:], 1.0, bvr[:, ch * 512:(ch + 1) * 512],
                            op0=ALU.mult, op1=ALU.add)
                        vt[ts][ch] = vtt

                    first = (c == 0 and ts == 0)
                    last = (c == NCHUNK - 1 and ts == NTS - 1)
                    for g in range(NCB):
                        ch, off = g // 4, (g % 4) * 128
                        cxp = ps_cx.tile([128, 128], f32, tag="cx", name="cxp")
                        nc.tensor.matmul(
                            cxp[:],
                            ke[ts][ch][:, off:off + 128],
                            vt[ts][ch][:, off:off + 128],
                            start=True, stop=True)
                        nc.vector.scalar_tensor_tensor(
                            ctxacc[g][:], cxp[:], 1.0, ctxacc[g][:],
                            op0=ALU.mult, op1=ALU.add)
                    for ch in range(2):
                        nc.tensor.matmul(
                            kcum[ch][:], onesc[:],
                            ke[ts][ch][:],
                            start=first, stop=last, skip_group_check=True)

            # ---------------- build ctxd / dg ----------------
            kcs = [consts.tile([1, 512], f32r, tag=f"kcs{i}", name=f"kcs{i}") for i in range(2)]
            for i in range(2):
                nc.vector.tensor_copy(kcs[i][:], kcum[i][:].bitcast(f32r))
            for g in range(NCB):
                kcsb = smallp.tile([128, 1], f32r, tag="kcsb")
                nc.sync.dma_start(
                    kcsb[:], kcs[g // 4][:, (g % 4) * 128:(g % 4) * 128 + 128])
                nc.vector.scalar_tensor_tensor(
                    dg[g][:], kcsb[:, 0:1].broadcast_to([128, 8]), 1.0, e8t[:],
                    op0=ALU.mult, op1=ALU.mult)
                nc.vector.scalar_tensor_tensor(
                    ctxd[g][:], ctxacc[g][:],
                    1.0, m128[:], op0=ALU.mult, op1=ALU.mult)

        # ---------------- pass 2 ----------------
        with ExitStack() as p2:
            wts2 = p2.enter_context(tc.tile_pool(name="wts2", bufs=1))
            qip = p2.enter_context(tc.tile_pool(name="qi", bufs=2))
            ynp = p2.enter_context(tc.tile_pool(name="yn", bufs=2))
            osp = p2.enter_context(tc.tile_pool(name="os", bufs=2))
            drp = p2.enter_context(tc.tile_pool(name="dr", bufs=2))
            ps_yt = p2.enter_context(
                tc.tile_pool(name="ps_yt", bufs=2, space="PSUM"))
            ps_dn = p2.enter_context(
                tc.tile_pool(name="ps_dn", bufs=2, space="PSUM"))
            ps_ex = p2.enter_context(
                tc.tile_pool(name="ps_ex", bufs=2, space="PSUM"))
            ps_o = p2.enter_context(
                tc.tile_pool(name="ps_o", bufs=2, space="PSUM"))

            wp = [wts2.tile([128, C], f32r, tag=f"wp{i}", name=f"wp{i}") for i in range(NCB)]
            for i in range(NCB):
                nc.sync.dma_start(wp[i][:], wp_d[i * 128:(i + 1) * 128, :].bitcast(f32r))

            for c in range(NCHUNK):
                t0 = c * TT
                qi = [qip.tile([128, TT], f32r, tag=f"qi{cb}", name=f"qi{cb}") for cb in range(NCB)]
                for cb in range(NCB):
                    nc.sync.dma_start(qi[cb][:], qspill[cb, :, t0:t0 + TT])
                yn = [None] * NCB
                for g in range(NCB):
                    ytp = ps_yt.tile([128, TT], f32, tag="yt")
                    nc.tensor.matmul(ytp[:], ctxd[g][:], qi[g][:],
                                     start=True, stop=True)
                    dnp = ps_dn.tile([8, TT], f32, tag="dn")
                    nc.tensor.matmul(dnp[:], dg[g][:], qi[g][:],
                                     start=True, stop=True)
                    dnr = drp.tile([8, TT], f32r, tag="dnr")
                    nc.vector.reciprocal(dnr[:], dnp[:])
                    exp = ps_ex.tile([128, TT], f32, tag="ex")
                    nc.tensor.matmul(exp[:], e8[:], dnr[:], start=True, stop=True)
                    exb = drp.tile([128, TT], f32, tag="exb")
                    nc.vector.tensor_copy(exb[:], exp[:])
                    ynt = ynp.tile([128, TT], f32r, tag=f"yn{g}")
                    nc.vector.scalar_tensor_tensor(
                        ynt[:], ytp[:], 1.0, exb[:], op0=ALU.mult, op1=ALU.mult)
                    yn[g] = ynt

                for ts in range(NTS):
                    for ch in range(2):
                        ops = ps_o.tile([128, 512], f32, tag="o")
                        for g in range(NCB):
                            nc.tensor.matmul(
                                ops[:], yn[g][:, ts * 128:(ts + 1) * 128],
                                wp[g][:, ch * 512:(ch + 1) * 512],
                                start=(g == 0), stop=(g == NCB - 1))
                        osb = osp.tile([128, 512], f32, tag=f"o{ts}{ch}")
                        nc.vector.scalar_tensor_tensor(
                            osb[:], ops[:], 1.0, bpr[:, ch * 512:(ch + 1) * 512],
                            op0=ALU.mult, op1=ALU.add)
                        nc.sync.dma_start(
                            out_d[t0 + ts * 128:t0 + (ts + 1) * 128,
                                  ch * 512:(ch + 1) * 512], osb[:])

    nc.compile()
    return nc


def _host_prep(Wq, bq, Wk, bk, Wv, bv, Wp, bp):
    return {
        "wqT": np.ascontiguousarray(Wq.T),
        "wkT": np.ascontiguousarray(Wk.T),
        "wvT": np.ascontiguousarray(Wv.T),
        "wpT": np.ascontiguousarray(Wp.T),
        "bq_cols": np.ascontiguousarray(np.asarray(bq).reshape(NCB, 128).T),
        "bk_rep": np.ascontiguousarray(np.tile(np.asarray(bk)[None, :], (128, 1))),
        "bv_rep": np.ascontiguousarray(np.tile(np.asarray(bv)[None, :], (128, 1))),
        "bp_rep": np.ascontiguousarray(np.tile(np.asarray(bp)[None, :], (128, 1))),
        "ident": np.eye(128, dtype=np.float32),
        "e8": np.repeat(np.eye(8, dtype=np.float32), 16, axis=1),
        "ones1": np.ones((1, 1), np.float32),
        "onescol": np.ones((128, 1), np.float32),
        "mask128": np.kron(np.eye(8, dtype=np.float32), np.ones((16, 16), np.float32)),
        "e8T": np.repeat(np.eye(8, dtype=np.float32), 16, axis=0),
    }


def kernel(x, Wq, bq, Wk, bk, Wv, bv, Wp, bp, _trace=False):
    if "nc" not in _cache:
        _cache["nc"] = build_nc()
    nc = _cache["nc"]
    shared = {k: np.asarray(v, np.float32) for k, v in
              _host_prep(Wq, bq, Wk, bk, Wv, bv, Wp, bp).items()}
    x = np.asarray(x, np.float32)
    in_maps = [dict(shared, x=np.ascontiguousarray(x[b])) for b in range(B)]
    res = run_bass_kernel_spmd(nc, in_maps, list(range(B)), trace=_trace)
    out = np.stack([res.results[b]["out"] for b in range(B)], axis=0)
    if _trace:
        _cache["last_results"] = res
    return out
